# revision 1
# baseline (speedup 1.0000x reference)
"""MultiHeadLiftLayer Trainium2 kernel.

reference:
    edge_signal = relu(x_0[src] @ W[:C] + x_0[tgt] @ W[C:])   # [E, 8]
    out = concat([edge_signal, x_1], axis=1)                   # [E, 72]

Strategy (8 NeuronCores, edges sharded):
  - Precompute per-node projections P_src = x_0 @ W[:C], P_tgt = x_0 @ W[C:]
    (each [N, 8]) on the tensor engine, stored as an f16 pair-packed table in
    SBUF: partition p holds one head-column (heads replicated; partitions
    0-63 = src heads, 64-127 = tgt heads), two consecutive nodes packed per
    u32 element -> num_elems 25000 fits ap_gather's int16-delta constraint.
  - Per 8192-edge call: GPSIMD ap_gather fetches the node pair for each
    edge (groups 0-3 use src indices of chunks 0-3, groups 4-7 tgt indices),
    DVE selects the even/odd f16 by node parity (host-provided u8 mask), and
    one PE matmul per 128-edge block against a fixed 0/1 selector sums the
    src/tgt lanes per head while landing directly in [edge, head] PSUM
    orientation. Rows are assembled in SBUF (p-major: partition p owns 64
    consecutive edges, so x_1 loads and output stores are one contiguous
    16-18KB DRAM run per partition) and stored with relu fused into the
    PSUM->SBUF copies.

    Measured on trn2: ap_gather runs ~28ns/idx (SBUF round-trip bound in the
    ucode, 4 idx per pipelined-depth-1 request); with 2 idx/edge spread over
    8 Q7 cores that is ~7ns/edge = ~545us for 78125 edges/core, which bounds
    the kernel; all DMA/PE/DVE/ACT work hides underneath it.
"""
import sys

sys.path.insert(0, "/opt/trn_rl_repo")

import numpy as np
import concourse.bass as bass
import concourse.tile as tile
from concourse import bacc, mybir
from concourse.bass_utils import run_bass_kernel_spmd

NUM_NODES = 50000
IN_CH0 = 128
HEADS = 8
NUM_EDGES = 625000
IN_CH1 = 64
OUT_CH = HEADS + IN_CH1  # 72

N_CORES = 8
E_CORE = NUM_EDGES // N_CORES  # 78125
L_MAIN = 2048                  # gather indices per call (per 16-partition group)
N_MAIN = 9                     # main calls: 9 * 4 * 2048 = 73728 edges
L_TAIL = 1152                  # tail call: 4 * 1152 = 4608 slots, 4397 valid
CALL_LS = [L_MAIN] * N_MAIN + [L_TAIL]
NPAIR = NUM_NODES // 2         # 25000 u32 elements per table column
NT = 2000                      # node-tile for the projection matmul
PCHUNK = 500                   # psum free-dim chunk

_cache = {}


def _build_program():
    if "nc" in _cache:
        return _cache["nc"]
    nc = bacc.Bacc("TRN2", target_bir_lowering=False, debug=False,
                   num_devices=N_CORES)
    f32, f16, i16 = mybir.dt.float32, mybir.dt.float16, mybir.dt.int16
    u8 = mybir.dt.uint8

    x0t = nc.dram_tensor("x0t", [IN_CH0, NUM_NODES], f16, kind="ExternalInput").ap()
    wbig = nc.dram_tensor("wbig", [IN_CH0, 128], f32, kind="ExternalInput").ap()
    x1 = nc.dram_tensor("x1", [E_CORE, IN_CH1], f32, kind="ExternalInput").ap()
    idx_in = nc.dram_tensor("idx", [len(CALL_LS), 128, L_MAIN // 16], i16,
                            kind="ExternalInput").ap()
    msel_in = nc.dram_tensor("msel", [128, 32], f32, kind="ExternalInput").ap()
    mask_in = nc.dram_tensor("mask", [len(CALL_LS), 128, L_MAIN], u8,
                             kind="ExternalInput").ap()
    out = nc.dram_tensor("out", [E_CORE, OUT_CH], f32, kind="ExternalOutput").ap()

    with tile.TileContext(nc) as tc:
        with tc.tile_pool(name="tab", bufs=1) as tab_pool, \
             tc.tile_pool(name="const", bufs=1) as const_pool:
            tab = tab_pool.tile([128, NPAIR], f32)       # f16 pair-packed view
            tab_f16 = tab[:].bitcast(f16)                # [128, 50000]
            msel32 = const_pool.tile([128, 32], f32)
            nc.sync.dma_start(msel32[:], msel_in[:])
            msel = const_pool.tile([128, 32], f16)
            nc.vector.tensor_copy(msel[:], msel32[:])

            # ---- phase 1: build the projection table ----
            with tc.tile_pool(name="p1", bufs=4) as p1_pool, \
                 tc.tile_pool(name="p1w", bufs=1) as p1w_pool, \
                 tc.tile_pool(name="p1ps", bufs=6, space="PSUM") as p1ps:
                wb32 = p1w_pool.tile([128, 128], f32)
                nc.sync.dma_start(wb32[:], wbig[:])
                wb16 = p1w_pool.tile([128, 128], f16)
                nc.vector.tensor_copy(wb16[:], wb32[:])
                for t in range(NUM_NODES // NT):
                    xt = p1_pool.tile([128, NT], f16, tag="xt")
                    nc.sync.dma_start(xt[:], x0t[:, t * NT:(t + 1) * NT])
                    for c in range(NT // PCHUNK):
                        ps = p1ps.tile([128, PCHUNK], f32)
                        nc.tensor.matmul(ps[:], lhsT=wb16[:],
                                         rhs=xt[:, c * PCHUNK:(c + 1) * PCHUNK],
                                         start=True, stop=True)
                        n0 = t * NT + c * PCHUNK
                        dst = tab_f16[:, n0:n0 + PCHUNK]
                        if c % 2 == 0:
                            nc.vector.tensor_copy(dst, ps[:])
                        else:
                            nc.scalar.copy(dst, ps[:])

            # ---- phase 2: gather / combine / emit ----
            with tc.tile_pool(name="io", bufs=3) as io_pool, \
                 tc.tile_pool(name="idxp", bufs=1) as idx_pool, \
                 tc.tile_pool(name="mega", bufs=2) as mega_pool, \
                 tc.tile_pool(name="p2ps", bufs=2, space="PSUM") as p2ps:
                its = []
                for k, L in enumerate(CALL_LS):
                    it = idx_pool.tile([128, L_MAIN // 16], i16, tag=f"it{k}")
                    nc.sync.dma_start(it[:, :L // 16], idx_in[k, :, :L // 16])
                    its.append(it)
                e_base = 0
                for k, L in enumerate(CALL_LS):
                    nseg = 4 * L // 128
                    it = its[k]
                    mk = io_pool.tile([128, L_MAIN], u8, tag="mk")
                    nc.sync.dma_start(mk[:, :L], mask_in[k, :, :L])

                    ot = io_pool.tile([128, L_MAIN], f32, tag="ot")
                    nc.gpsimd.ap_gather(out_ap=ot[:, :L], in_ap=tab[:],
                                        idxs_ap=it[:, :L // 16], channels=128,
                                        num_elems=NPAIR, d=1, num_idxs=L)
                    pair = ot[:, :L].bitcast(f16).rearrange(
                        "p (l two) -> p l two", two=2)
                    sel = io_pool.tile([128, L_MAIN], f16, tag="sel")
                    nc.vector.tensor_copy(sel[:, :L], pair[:, :, 0])
                    nc.vector.copy_predicated(sel[:, :L], mk[:, :L], pair[:, :, 1])

                    # per 128-edge block: one PE matmul sums the src lane
                    # and tgt lane per head (fixed 0/1 selector as the moving
                    # operand) and lands directly in [edge, head] orientation:
                    # psum[e, 8g+h] = sel[16g+h, e] + sel[64+16g+h, e]
                    nb = L // 128
                    ps2 = p2ps.tile([128, 512], f32)
                    for b in range(nb):
                        nc.tensor.matmul(ps2[:, 32 * b:32 * b + 32],
                                         lhsT=sel[:, 128 * b:128 * (b + 1)],
                                         rhs=msel[:], start=True, stop=True)

                    mega = mega_pool.tile([128, 64, OUT_CH], f32)
                    # relu fused into the PSUM->SBUF copies; chunk g block b
                    # sits at psum cols [32b + 8g : +8], destination seg g*nb+b
                    psv = ps2[:, :32 * nb].rearrange("p (s h) -> p s h", h=32)
                    for g in range(4):
                        nc.scalar.activation(
                            mega[:, g * nb:(g + 1) * nb, :HEADS],
                            psv[:, :, 8 * g:8 * g + 8],
                            mybir.ActivationFunctionType.Relu)

                    if k < N_MAIN:
                        # p-major: partition p holds edges [e_base+64p, +64),
                        # giving one contiguous 16-18KB DRAM run per partition
                        v = slice(e_base, e_base + 4 * L)
                        nc.sync.dma_start(
                            mega[:, :, HEADS:],
                            x1[v].rearrange("(p s) c -> p s c", s=64))
                        nc.scalar.dma_start(
                            out[v].rearrange("(p s) c -> p s c", s=64),
                            mega[:])
                    else:
                        # tail: seg-major with partial coverage
                        n_edges = min(E_CORE - e_base, 4 * L)
                        full_seg = n_edges // 128
                        rem = n_edges - full_seg * 128
                        if full_seg:
                            v = slice(e_base, e_base + full_seg * 128)
                            nc.sync.dma_start(
                                mega[:, :full_seg, HEADS:],
                                x1[v].rearrange("(s p) c -> p s c", p=128))
                            nc.scalar.dma_start(
                                out[v].rearrange("(s p) c -> p s c", p=128),
                                mega[:, :full_seg, :])
                        if rem:
                            v = slice(e_base + full_seg * 128, e_base + n_edges)
                            nc.sync.dma_start(mega[:rem, full_seg, HEADS:], x1[v])
                            nc.scalar.dma_start(out[v], mega[:rem, full_seg, :])
                    e_base += 4 * L

    nc.compile()
    _cache["nc"] = nc
    return nc


def _prep_inputs(x_0, adjacency_0, x_1, att_parameter):
    x0t = np.ascontiguousarray(np.asarray(x_0).T).astype(np.float16)
    wbig = np.empty((IN_CH0, 128), np.float32)
    for p in range(128):
        half = IN_CH0 * (p >= 64)
        wbig[:, p] = att_parameter[half:half + IN_CH0, p % 8]

    msel = np.zeros((128, 32), np.float32)
    for g in range(4):
        for h in range(8):
            msel[16 * g + h, 8 * g + h] = 1.0
            msel[64 + 16 * g + h, 8 * g + h] = 1.0

    src_all = np.asarray(adjacency_0[0]).astype(np.int64)
    tgt_all = np.asarray(adjacency_0[1]).astype(np.int64)
    x_1 = np.asarray(x_1, dtype=np.float32)

    in_maps = []
    for core in range(N_CORES):
        lo = core * E_CORE
        src = src_all[lo:lo + E_CORE]
        tgt = tgt_all[lo:lo + E_CORE]
        idx_a = np.zeros((len(CALL_LS), 128, L_MAIN // 16), np.int16)
        mask_a = np.zeros((len(CALL_LS), 128, L_MAIN), np.uint8)
        e = 0
        pos = np.arange(L_MAIN)
        pmaj = 64 * (pos % 128) + (pos // 128)  # i = 128b+p -> 64p + b
        for k, L in enumerate(CALL_LS):
            for g in range(4):
                if k < N_MAIN:
                    eoff = e + pmaj + 16 * g
                    sv = src[eoff]
                    tv = tgt[eoff]
                else:
                    c0 = e + g * L
                    sv = src[c0:c0 + L]
                    tv = tgt[c0:c0 + L]
                    if len(sv) < L:  # tail padding
                        sv = np.concatenate([sv, np.zeros(L - len(sv), np.int64)])
                        tv = np.concatenate([tv, np.zeros(L - len(tv), np.int64)])
                # wrapped: idxs[p, s] = v[16 s + p]
                idx_a[k, 16 * g:16 * g + 16, :L // 16] = \
                    (sv >> 1).astype(np.int16).reshape(L // 16, 16).T
                idx_a[k, 64 + 16 * g:64 + 16 * g + 16, :L // 16] = \
                    (tv >> 1).astype(np.int16).reshape(L // 16, 16).T
                mask_a[k, 16 * g:16 * g + 16, :L] = \
                    (sv & 1).astype(np.uint8)[None, :]
                mask_a[k, 64 + 16 * g:64 + 16 * g + 16, :L] = \
                    (tv & 1).astype(np.uint8)[None, :]
            e += 4 * L
        in_maps.append({
            "x0t": x0t,
            "wbig": wbig,
            "msel": msel,
            "x1": x_1[lo:lo + E_CORE],
            "idx": idx_a,
            "mask": mask_a,
        })
    return in_maps


def kernel(x_0, adjacency_0, x_1, att_parameter, _trace=False):
    # materialize as numpy up front: slicing jax arrays here would trigger
    # device jit compiles of generic XLA ops, which this toolchain rejects
    x_0 = np.asarray(x_0, dtype=np.float32)
    adjacency_0 = np.asarray(adjacency_0)
    x_1 = np.asarray(x_1, dtype=np.float32)
    att_parameter = np.asarray(att_parameter, dtype=np.float32)
    nc = _build_program()
    in_maps = _prep_inputs(x_0, adjacency_0, x_1, att_parameter)
    res = run_bass_kernel_spmd(nc, in_maps, list(range(N_CORES)), trace=_trace)
    out = np.concatenate([res.results[i]["out"] for i in range(N_CORES)], axis=0)
    kernel.last_exec_time_ns = res.exec_time_ns
    return out



# revision 2
# speedup vs baseline: 1.0232x; 1.0232x over previous
"""MultiHeadLiftLayer Trainium2 kernel, v2: edge-pairing gather.

Baseline machinery (pair-packed f16 projection table in SBUF, GPSIMD
ap_gather, DVE parity select, per-128-slot PE matmul with a fixed 0/1
selector), plus host-side edge pairing: the table's node order per core and
per side is a free host choice, so two edges whose src nodes share a packed
u32 (and whose tgt nodes share one too) are served by ONE gather index.
Each index yields two output columns; a per-column parity mask selects which
f16 of the gathered u32 each column takes.

Pairing passes: (A) same-src edges pair (src key = any key holding the node;
both columns take the same parity), (B) same-tgt among leftovers, then the
rest ride as singles (one index, one live column). Index count per side
drops from E to E - #pairs (~0.64E).
"""
import sys

sys.path.insert(0, "/opt/trn_rl_repo")

import numpy as np
import concourse.bass as bass
import concourse.tile as tile
from concourse import bacc, mybir
from concourse.bass_utils import run_bass_kernel_spmd

NUM_NODES = 50000
IN_CH0 = 128
HEADS = 8
NUM_EDGES = 625000
IN_CH1 = 64
OUT_CH = HEADS + IN_CH1  # 72

N_CORES = 8
E_CORE = NUM_EDGES // N_CORES  # 78125
L = 1024                        # gather indices per call per Q7 group
COLS = 2 * L                    # output columns per chunk per call
SLOTS_CALL = 4 * COLS           # 8192 slots per call
NB = COLS // 128                # 16 psum blocks
NT = 2000                       # f16 cols per phase-1 tile
PCHUNK = 500
KEY_CAP = 32768

_cache = {}


def _match_core(src, tgt):
    """Returns (units, m_s, m_t, par_s, par_t, pairs_s, pairs_t).
    units: list of (eA, eB) with eB=-1 for singles. m_s/m_t: per-unit table
    index per side. par_s/par_t: per-unit (parA, parB) column parities.
    pairs_s/pairs_t: [K, 2] node contents of each table pair (-1 = hole).
    """
    E = len(src)
    used = np.zeros(E, bool)
    units = []

    by = {}
    for e in range(E):
        by.setdefault(src[e], []).append(e)
    pairsA = []  # same-src units
    for s, es in by.items():
        while len(es) >= 2:
            e1, e2 = es.pop(), es.pop()
            used[e1] = used[e2] = True
            pairsA.append((e1, e2))
    by = {}
    for e in range(E):
        if not used[e]:
            by.setdefault(tgt[e], []).append(e)
    pairsB = []  # same-tgt units
    for t, es in by.items():
        while len(es) >= 2:
            e1, e2 = es.pop(), es.pop()
            used[e1] = used[e2] = True
            pairsB.append((e1, e2))
    singles = [e for e in range(E) if not used[e]]

    # --- key allocation per side ---
    class Side:
        def __init__(self):
            self.pairs = []      # list of [a, b]
            self.key = {}        # unordered frozen pair -> idx
            self.slot = {}       # node -> (key_idx, parity)

        def alloc_pair(self, a, b):
            k = (a, b) if a <= b else (b, a)
            i = self.key.get(k)
            if i is None:
                i = len(self.pairs)
                self.pairs.append([k[0], k[1]])
                self.key[k] = i
                self.slot.setdefault(k[0], (i, 0))
                self.slot.setdefault(k[1], (i, 1))
            return i

        def need(self, n, pend):
            if n not in self.slot and n not in pend:
                pend[n] = True

        def pack(self, pend):
            ns = [n for n in pend if n not in self.slot]
            for i in range(0, len(ns) - 1, 2):
                self.alloc_pair(ns[i], ns[i + 1])
            if len(ns) % 2:
                self.alloc_pair(ns[-1], ns[-1])

    S, T = Side(), Side()
    # explicit pair keys
    for e1, e2 in pairsA:
        T.alloc_pair(tgt[e1], tgt[e2])
    for e1, e2 in pairsB:
        S.alloc_pair(src[e1], src[e2])
    # packing for membership-only needs
    pend_s, pend_t = {}, {}
    for e1, e2 in pairsA:
        S.need(src[e1], pend_s)
    for e1, e2 in pairsB:
        T.need(tgt[e1], pend_t)
    for e in singles:
        S.need(src[e], pend_s)
        T.need(tgt[e], pend_t)
    S.pack(pend_s)
    T.pack(pend_t)

    m_s, m_t, par_s, par_t = [], [], [], []

    def emit(eA, eB):
        units.append((eA, eB))
        if eB >= 0 and src[eA] == src[eB]:
            i, p = S.slot[src[eA]]
            m_s.append(i)
            par_s.append((p, p))
        elif eB >= 0:
            i = S.alloc_pair(src[eA], src[eB])
            a, b = S.pairs[i]
            m_s.append(i)
            par_s.append((0 if src[eA] == a else 1, 1 if src[eB] == b else 0))
        else:
            i, p = S.slot[src[eA]]
            m_s.append(i)
            par_s.append((p, 0))
        if eB >= 0 and tgt[eA] == tgt[eB]:
            i, p = T.slot[tgt[eA]]
            m_t.append(i)
            par_t.append((p, p))
        elif eB >= 0:
            i = T.alloc_pair(tgt[eA], tgt[eB])
            a, b = T.pairs[i]
            m_t.append(i)
            par_t.append((0 if tgt[eA] == a else 1, 1 if tgt[eB] == b else 0))
        else:
            i, p = T.slot[tgt[eA]]
            m_t.append(i)
            par_t.append((p, 0))

    for e1, e2 in pairsA:
        emit(e1, e2)
    for e1, e2 in pairsB:
        emit(e1, e2)
    for e in singles:
        emit(e, -1)

    assert len(S.pairs) <= KEY_CAP and len(T.pairs) <= KEY_CAP, \
        (len(S.pairs), len(T.pairs))
    return (units, np.array(m_s), np.array(m_t), np.array(par_s),
            np.array(par_t), np.array(S.pairs), np.array(T.pairs))


def _prep(x_0, adjacency_0, x_1, att_parameter):
    src_all = np.asarray(adjacency_0[0]).astype(np.int64)
    tgt_all = np.asarray(adjacency_0[1]).astype(np.int64)
    x_1 = np.asarray(x_1, dtype=np.float32)
    x0f16 = np.asarray(x_0, dtype=np.float32).astype(np.float16)

    wbig = np.empty((IN_CH0, 128), np.float32)
    for p in range(128):
        half = IN_CH0 * (p >= 64)
        wbig[:, p] = att_parameter[half:half + IN_CH0, p % 8]

    msel = np.zeros((128, 32), np.float32)
    for g in range(4):
        for h in range(8):
            msel[16 * g + h, 8 * g + h] = 1.0
            msel[64 + 16 * g + h, 8 * g + h] = 1.0

    cores = []
    umax = kmax = 0
    for core in range(N_CORES):
        lo = core * E_CORE
        r = _match_core(src_all[lo:lo + E_CORE], tgt_all[lo:lo + E_CORE])
        cores.append(r)
        umax = max(umax, len(r[0]))
        kmax = max(kmax, len(r[5]), len(r[6]))
    ncalls = -(-umax // (4 * L))
    s_core = ncalls * SLOTS_CALL
    npair = min(KEY_CAP, -(-kmax // 250) * 250)

    in_maps, slot_maps = [], []
    for core in range(N_CORES):
        lo = core * E_CORE
        units, m_s, m_t, par_s, par_t, pairs_s, pairs_t = cores[core]
        U = len(units)

        x0t2 = np.zeros((128, 2, npair, 2), np.float16)  # [c, side, pair, par]
        for half, pr in ((0, pairs_s), (1, pairs_t)):
            a, b = pr[:, 0], pr[:, 1]
            n = len(a)
            x0t2[:, half, :n, 0][:, a >= 0] = x0f16[a[a >= 0]].T
            x0t2[:, half, :n, 1][:, b >= 0] = x0f16[b[b >= 0]].T
        x0t = np.ascontiguousarray(x0t2.reshape(128, 4 * npair))

        grid = 4 * L * ncalls
        mseq = np.zeros(grid, np.int64)
        mtseq = np.zeros(grid, np.int64)
        mseq[:U] = m_s
        mtseq[:U] = m_t
        # masks: [ncalls, 128, COLS] u8; group g rows 16g..16g+16 share the
        # chunk's per-column parity (src side), rows 64+16g.. the tgt side
        pseq = np.zeros((grid, 2), np.uint8)
        tseq = np.zeros((grid, 2), np.uint8)
        pseq[:U] = par_s
        tseq[:U] = par_t
        idx_a = np.zeros((ncalls, 128, L // 16), np.int16)
        mask_a = np.zeros((ncalls, 128, COLS), np.uint8)
        for k in range(ncalls):
            for g in range(4):
                b0 = (k * 4 + g) * L
                w_s = mseq[b0:b0 + L].reshape(L // 16, 16).T.astype(np.int16)
                w_t = mtseq[b0:b0 + L].reshape(L // 16, 16).T.astype(np.int16)
                idx_a[k, 16 * g:16 * g + 16, :] = w_s
                idx_a[k, 64 + 16 * g:64 + 16 * g + 16, :] = w_t
                # column c = 2j + r ; stream position j wraps as 16s+i -> but
                # columns are consumed j-major: unit at stream pos j covers
                # cols 2j, 2j+1 with j = 16*s + i?  No: ap_gather output col
                # j holds index stream position j = 16*s + i where
                # idx[16g+i, s]. Masks are per OUTPUT column: unit u sits at
                # output position j_out = its position in the L-stream.
                ms = pseq[b0:b0 + L].reshape(L, 2)   # unit j -> (parA, parB)
                mt_ = tseq[b0:b0 + L].reshape(L, 2)
                cols_s = np.empty(COLS, np.uint8)
                cols_t = np.empty(COLS, np.uint8)
                cols_s[0::2] = ms[:, 0]
                cols_s[1::2] = ms[:, 1]
                cols_t[0::2] = mt_[:, 0]
                cols_t[1::2] = mt_[:, 1]
                mask_a[k, 16 * g:16 * g + 16, :] = cols_s[None, :]
                mask_a[k, 64 + 16 * g:64 + 16 * g + 16, :] = cols_t[None, :]

        # unit u -> stream (k, g, j): u = (k*4 + g)*L + j  (j-major fill)
        u_ids = np.arange(U)
        k_arr = u_ids // (4 * L)
        g_arr = (u_ids // L) % 4
        j_arr = u_ids % L
        slot_of_edge = np.full(E_CORE, -1, np.int64)
        eA = np.array([u[0] for u in units])
        eB = np.array([u[1] for u in units])
        for par, ee in ((0, eA), (1, eB)):
            c = 2 * j_arr + par
            b = c // 128
            p = c % 128
            slot = SLOTS_CALL * k_arr + 64 * p + 16 * g_arr + b
            v = ee >= 0
            slot_of_edge[ee[v]] = slot[v]
        assert (slot_of_edge >= 0).all()

        x1s = np.zeros((s_core, IN_CH1), np.float32)
        x1s[slot_of_edge] = x_1[lo:lo + E_CORE]

        in_maps.append({
            "x0t": x0t,
            "wbig": wbig,
            "msel": msel,
            "x1": x1s,
            "idx": idx_a,
            "mask": mask_a,
        })
        slot_maps.append(slot_of_edge)
    return in_maps, slot_maps, npair, ncalls, s_core


def _build_program(npair, ncalls, s_core):
    key = ("nc", npair, ncalls)
    if key in _cache:
        return _cache[key]
    nc = bacc.Bacc("TRN2", target_bir_lowering=False, debug=False,
                   num_devices=N_CORES)
    f32, f16, i16 = mybir.dt.float32, mybir.dt.float16, mybir.dt.int16
    u8 = mybir.dt.uint8

    x0t = nc.dram_tensor("x0t", [IN_CH0, 4 * npair], f16, kind="ExternalInput").ap()
    wbig = nc.dram_tensor("wbig", [IN_CH0, 128], f32, kind="ExternalInput").ap()
    x1 = nc.dram_tensor("x1", [s_core, IN_CH1], f32, kind="ExternalInput").ap()
    idx_in = nc.dram_tensor("idx", [ncalls, 128, L // 16], i16,
                            kind="ExternalInput").ap()
    mask_in = nc.dram_tensor("mask", [ncalls, 128, COLS], u8,
                             kind="ExternalInput").ap()
    msel_in = nc.dram_tensor("msel", [128, 32], f32, kind="ExternalInput").ap()
    out = nc.dram_tensor("out", [s_core, OUT_CH], f32, kind="ExternalOutput").ap()

    with tile.TileContext(nc) as tc:
        with tc.tile_pool(name="tab", bufs=1) as tab_pool, \
             tc.tile_pool(name="const", bufs=1) as const_pool:
            tab = tab_pool.tile([128, npair], f32)
            tab_f16 = tab[:].bitcast(f16)            # [128, 2*npair]
            msel32 = const_pool.tile([128, 32], f32)
            nc.sync.dma_start(msel32[:], msel_in[:])
            mselt = const_pool.tile([128, 32], f16)
            nc.vector.tensor_copy(mselt[:], msel32[:])

            # ---- phase 1: both halves of the projection table ----
            ncols = 2 * npair
            with tc.tile_pool(name="p1", bufs=4) as p1_pool, \
                 tc.tile_pool(name="p1w", bufs=1) as p1w_pool, \
                 tc.tile_pool(name="p1ps", bufs=6, space="PSUM") as p1ps:
                wb32 = p1w_pool.tile([128, 128], f32)
                nc.sync.dma_start(wb32[:], wbig[:])
                wb16 = p1w_pool.tile([128, 128], f16)
                nc.vector.tensor_copy(wb16[:], wb32[:])
                for t in range(-(-ncols // NT)):
                    c0 = t * NT
                    cw = min(NT, ncols - c0)
                    xs = p1_pool.tile([128, NT], f16, tag="xs")
                    nc.sync.dma_start(xs[:, :cw], x0t[:, c0:c0 + cw])
                    xt = p1_pool.tile([128, NT], f16, tag="xt")
                    nc.sync.dma_start(xt[:, :cw], x0t[:, ncols + c0:ncols + c0 + cw])
                    for c in range(-(-cw // PCHUNK)):
                        cl = min(PCHUNK, cw - c * PCHUNK)
                        ps = p1ps.tile([128, PCHUNK], f32)
                        nc.tensor.matmul(ps[0:64, :cl], lhsT=wb16[:, 0:64],
                                         rhs=xs[:, c * PCHUNK:c * PCHUNK + cl],
                                         start=True, stop=True)
                        nc.tensor.matmul(ps[64:128, :cl], lhsT=wb16[:, 64:128],
                                         rhs=xt[:, c * PCHUNK:c * PCHUNK + cl],
                                         start=True, stop=True)
                        dst = tab_f16[:, c0 + c * PCHUNK:c0 + c * PCHUNK + cl]
                        if c % 2 == 0:
                            nc.vector.tensor_copy(dst, ps[:, :cl])
                        else:
                            nc.scalar.copy(dst, ps[:, :cl])

            # ---- phase 2 ----
            with tc.tile_pool(name="io", bufs=2) as io_pool, \
                 tc.tile_pool(name="idxp", bufs=1) as idx_pool, \
                 tc.tile_pool(name="mega", bufs=2) as mega_pool, \
                 tc.tile_pool(name="p2ps", bufs=2, space="PSUM") as p2ps:
                its = []
                for k in range(ncalls):
                    it = idx_pool.tile([128, L // 16], i16, tag=f"it{k}")
                    nc.sync.dma_start(it[:], idx_in[k])
                    its.append(it)
                for k in range(ncalls):
                    mk = io_pool.tile([128, COLS], u8, tag="mk")
                    nc.sync.dma_start(mk[:], mask_in[k])
                    ot = io_pool.tile([128, L], f32, tag="ot")
                    nc.gpsimd.ap_gather(out_ap=ot[:], in_ap=tab[:],
                                        idxs_ap=its[k][:], channels=128,
                                        num_elems=npair, d=1, num_idxs=L)
                    pair = ot[:].bitcast(f16).rearrange(
                        "p (l two) -> p l two", two=2)
                    sel = io_pool.tile([128, COLS], f16, tag="sel")
                    selv = sel[:].rearrange("p (l two) -> p l two", two=2)
                    mkv = mk[:].rearrange("p (l two) -> p l two", two=2)
                    for r in range(2):
                        nc.vector.tensor_copy(selv[:, :, r], pair[:, :, 0])
                        nc.vector.copy_predicated(selv[:, :, r], mkv[:, :, r],
                                                  pair[:, :, 1])

                    ps2 = p2ps.tile([128, 512], f32)
                    for b in range(NB):
                        nc.tensor.matmul(ps2[:, 32 * b:32 * b + 32],
                                         lhsT=sel[:, 128 * b:128 * (b + 1)],
                                         rhs=mselt[:], start=True, stop=True)

                    mega = mega_pool.tile([128, 64, OUT_CH], f32)
                    psv = ps2[:].rearrange("p (s h) -> p s h", h=32)
                    for g in range(4):
                        nc.scalar.activation(
                            mega[:, g * NB:(g + 1) * NB, :HEADS],
                            psv[:, :, 8 * g:8 * g + 8],
                            mybir.ActivationFunctionType.Relu)
                    v = slice(k * SLOTS_CALL, (k + 1) * SLOTS_CALL)
                    nc.sync.dma_start(
                        mega[:, :, HEADS:],
                        x1[v].rearrange("(p s) c -> p s c", s=64))
                    nc.scalar.dma_start(
                        out[v].rearrange("(p s) c -> p s c", s=64),
                        mega[:])

    nc.compile()
    _cache[key] = nc
    return nc


def kernel(x_0, adjacency_0, x_1, att_parameter, _trace=False):
    x_0 = np.asarray(x_0, dtype=np.float32)
    adjacency_0 = np.asarray(adjacency_0)
    x_1 = np.asarray(x_1, dtype=np.float32)
    att_parameter = np.asarray(att_parameter, dtype=np.float32)
    in_maps, slot_maps, npair, ncalls, s_core = _prep(
        x_0, adjacency_0, x_1, att_parameter)
    nc = _build_program(npair, ncalls, s_core)
    res = run_bass_kernel_spmd(nc, in_maps, list(range(N_CORES)), trace=_trace)
    outs = []
    for core in range(N_CORES):
        outs.append(res.results[core]["out"][slot_maps[core]])
    kernel.last_exec_time_ns = res.exec_time_ns
    return np.concatenate(outs, axis=0)
